# revision 25
# baseline (speedup 1.0000x reference)
"""Trainium2 Bass kernel for nn_ColRepeatCausalLinear.

Math: reference computes out = x @ W + bias with
    W[s, t] = v[t] * d^(t-s)  for t >= s, else 0,   d = clip(decay_value, 0.9, 1)
which factorizes as a decayed prefix scan along S:
    y[b, e, t] = d * y[b, e, t-1] + x[b, e, t]
    out[b, e, t] = v[t] * y[b, e, t] + bias[t]
i.e. O(B*E*S) work instead of the O(B*E*S^2) dense matmul.

Mapping: data-parallel over B across 8 NeuronCores (x[b] per core, params
replicated). Per core the kernel sits on the DMA wall: 8 MiB in + 8 MiB
out against a measured ~428 GB/s aggregate DMA fabric (16 engines), so
wall clock ~= (first-byte latency ~8.6us, fixed NEFF preamble + queue
arm) + 16.4 MiB / 428 GB/s + ~2.5us counted epilogue. Structure to stay
at the wall (all measured on HW):
  - x moves in eight [128, S] tiles; each is ONE fully DRAM-contiguous
    1 MiB HWDGE group of exactly 128 x 8 KiB descriptors. Only
    exact-128-descriptor groups spread evenly over all 16 DMA engines
    (64-desc groups ran at half rate; 113/15-desc splits collapsed onto
    one engine, 11x slower).
  - load groups alternate between the two HWDGE queues (SP/Act) and
    each queue gets 4 load + 4 store groups: a queue only sustains ~4
    outstanding trigger groups (a 5th dma_start stalls the issuing
    engine), and alternating makes scan i gate on queue (i%2)'s ~4.7
    us/group pace, so the scan chain advances every ~2.35 us.
  - all load triggers enqueue before any store trigger on both rings
    (an event-gated store descriptor never head-of-line blocks a load);
    each tile's store rides the queue opposite its load, keeping both
    rings supplied through the load->store transition.
  - v is host-cast to bf16 and broadcast across partitions with a K=1
    ones-matmul into PSUM (bf16 moving data runs the PE at 1 cyc/row vs
    4 for fp32, so vb is ready ~2 us after v lands, off the scan gate);
  - the scan+scale runs per e-row tile on the Vector engine via a fused
    custom DVE op (cumsum * v in one pass, ~1 cyc/elem, 2.29us/tile).
Known residual: one DMA engine (ring 15 / engine 79) intermittently
runs ~60% speed; round-robin descriptor assignment has no work
stealing, so such runs straggle ~7-9us draining its share at the end.
No descriptor layout avoids it (see above); this is the bimodality
between ~52.5us and ~60us runs.

Hardcoded problem shapes: x (8, 1024, 2048) f32, weight (1, 2048),
bias (2048,), decay_value (1,).
"""

import numpy as np

import concourse.bacc as bacc
import concourse.mybir as mybir
from concourse.tile import TileContext
from concourse.bass_utils import run_bass_kernel_spmd

B, E, S = 8, 1024, 2048
P = 128
N_CORES = 8
F32 = mybir.dt.float32
BF16 = mybir.dt.bfloat16

_cache = {}

# Fused custom DVE op: out[p,k] = (sum_{j<=k} x[p,j]) * v[p,k] — the whole
# d=1 kernel body in ONE Vector-engine instruction (the stock path needs a
# 2-cyc/elem TensorTensorScan plus a 1-cyc/elem tensor_mul). Registered at
# runtime into dve_ops.OPS; sha self-pinned since this op isn't in-tree.
_FUSED_OP = None
try:
    from concourse import dve_ops as _dops
    from concourse.dve_spec import AluOp as _AluOp, Spec as _Spec
    from concourse.dve_spec import Src0 as _Src0, Src1 as _Src1, scan as _scan
    from concourse.dve_spec import lower as _lower
    from concourse.dve_uop import DveOpSpec as _DveOpSpec

    _FUSED_NAME = "CUMSUM_VSCALE_ANT"
    if _FUSED_NAME in _dops._SUB_OPCODE_FOR_NAME:
        _FUSED_OP = next(o for o in _dops.OPS if o.name == _FUSED_NAME)
    else:
        _fspec = _Spec(body=_scan(_AluOp.ADD, _Src0) * _Src1)
        _row = _dops._CUSTOM_DVE_ROW_BASE + len(_dops.OPS)
        assert _row < 0x20
        _dops._SUB_OPCODE_FOR_NAME[_FUSED_NAME] = _row
        _sha = {}
        for _ver in ("v3", "v4"):
            try:
                _sha[_ver] = _DveOpSpec(
                    name=_FUSED_NAME,
                    opcode=_row,
                    uops=_lower(_fspec, ver=_ver),
                    rd1_en=_dops.has_src1(_fspec),
                ).sha(_ver)
            except Exception:
                pass
        _FUSED_OP = _dops.DveOp(_FUSED_NAME, _fspec, subdim=False, uops_sha=_sha)
        _dops.OPS.append(_FUSED_OP)
        _dops.CUSTOM_DVE_SPECS[_FUSED_NAME] = _fspec
except Exception:
    _FUSED_OP = None

R = 1  # e-rows per partition per tile
BANK = 512  # fp32 elems per PSUM bank


def _build(d: float, has_bias: bool):
    nc = bacc.Bacc(
        "TRN2",
        target_bir_lowering=False,
        debug=False,
        enable_asserts=False,
    )
    x = nc.dram_tensor("x", [E, S], F32, kind="ExternalInput").ap()
    v_dram = nc.dram_tensor("v", [1, S], BF16, kind="ExternalInput").ap()
    bias_dram = None
    if has_bias:
        bias_dram = nc.dram_tensor("bias", [1, S], F32, kind="ExternalInput").ap()
    out = nc.dram_tensor("out", [E, S], F32, kind="ExternalOutput").ap()

    n_tiles = E // (P * R)
    rows = P * R

    with TileContext(nc) as tc:
        with (
            tc.tile_pool(name="const", bufs=1) as cpool,
            tc.tile_pool(name="xs", bufs=n_tiles) as xpool,
            tc.tile_pool(name="ys", bufs=2) as ypool,
            tc.tile_pool(name="os", bufs=n_tiles) as opool,
            tc.tile_pool(name="ps", bufs=1, space="PSUM") as ppool,
        ):
            # v (4 KiB bf16) rides the gpsimd SWDGE queue: slow to start
            # (~13 us arrival) but OFF both HWDGE rings — a leading v
            # group on a bulk ring time-shifts that whole ring by the
            # ~2 us group-handoff latency and made it finish last. With
            # the bf16 PE broadcast (~1.5 us), vb is still ready ~15 us,
            # ahead of every scan/store deadline.
            vrow = cpool.tile([1, S], BF16)
            nc.gpsimd.dma_start(out=vrow[:], in_=v_dram)
            if has_bias:
                brow = cpool.tile([1, S], F32)
                nc.gpsimd.dma_start(out=brow[:], in_=bias_dram)

            # x loads: one fully DRAM-contiguous 1 MiB group (128 x 8 KiB
            # descriptors — only exact-128 groups spread evenly over all 16
            # DMA engines) per [128, S] tile, alternating queues so scan i
            # gates on queue (i%2)'s ~4.7 us/group pace — interleaved, the
            # chain advances every ~2.35 us. 4 load + 4 store groups per
            # queue matches the ~4-outstanding-group HWDGE trigger limit.
            xts = []
            for i in range(n_tiles):
                xt = xpool.tile([P, R * S], F32)
                src = x[i * rows : (i + 1) * rows, :].rearrange(
                    "(p b) s -> p (b s)", b=R
                )
                (nc.scalar if i % 2 == 0 else nc.sync).dma_start(out=xt[:], in_=src)
                xts.append(xt)

            # Broadcast v across partitions with a K=1 matmul against a
            # ones row (out[p, t] = v[t]); bf16 moving data streams the PE
            # at 1 cyc/row. The Vector engine reads vb from PSUM.
            ones = cpool.tile([1, P], BF16)
            nc.vector.memset(ones[:], 1.0)
            vb = ppool.tile([P, S], F32)
            for n in range(S // BANK):
                nc.tensor.matmul(
                    vb[:, n * BANK : (n + 1) * BANK],
                    ones[:],
                    vrow[:, n * BANK : (n + 1) * BANK],
                    start=True,
                    stop=True,
                )
            if has_bias:
                onesf = cpool.tile([1, P], F32)
                nc.vector.memset(onesf[:], 1.0)
                bb = ppool.tile([P, S], F32)
                for n in range(S // BANK):
                    nc.tensor.matmul(
                        bb[:, n * BANK : (n + 1) * BANK],
                        onesf[:],
                        brow[:, n * BANK : (n + 1) * BANK],
                        start=True,
                        stop=True,
                    )
            if not (d == 1.0 and _FUSED_OP is not None):
                dtile = cpool.tile([P, 1], F32)
                nc.gpsimd.memset(dtile[:], d)

            for i in range(n_tiles):
                xt = xts[i]
                ot = opool.tile([P, R * S], F32)
                dst = out[i * rows : (i + 1) * rows, :].rearrange(
                    "(p b) s -> p (b s)", b=R
                )
                for c in range(R):
                    cs = slice(c * S, (c + 1) * S)
                    xc = xt[:, cs]
                    oc = ot[:, cs]
                    if d == 1.0 and _FUSED_OP is not None:
                        nc.vector._custom_dve(_FUSED_OP, out=oc, in0=xc, in1=vb[:])
                    else:
                        yt = ypool.tile([P, S], F32)
                        nc.vector.tensor_tensor_scan(
                            yt[:], dtile[:].broadcast_to([P, S]), xc,
                            0.0, mybir.AluOpType.mult, mybir.AluOpType.add,
                        )
                        nc.vector.tensor_mul(oc, yt[:], vb[:])
                    if has_bias:
                        nc.vector.tensor_add(oc, oc, bb[:])
                # Store the tile as one 1 MiB group on the queue
                # opposite its load.
                (nc.sync if i % 2 == 0 else nc.scalar).dma_start(
                    out=dst, in_=ot[:]
                )
    nc.compile()
    return nc


def _run(x, weight, bias, decay_value, trace=False):
    x = np.asarray(x, dtype=np.float32)
    weight = np.asarray(weight, dtype=np.float32)
    bias = np.asarray(bias, dtype=np.float32)
    decay_value = np.asarray(decay_value)
    assert x.shape == (B, E, S), x.shape

    # DECAY_CONSTANT = 1.0 in the reference; exponent is (t - s) / 1.0.
    d = float(np.clip(np.float64(decay_value.reshape(-1)[0]), 0.9, 1.0))
    has_bias = bool(np.any(bias))

    key = (d, has_bias)
    if key not in _cache:
        _cache[key] = _build(d, has_bias)
    nc = _cache[key]

    import ml_dtypes

    vrow = np.ascontiguousarray(
        weight.reshape(1, S).astype(ml_dtypes.bfloat16)
    )
    in_maps = []
    for b in range(N_CORES):
        m = {"x": np.ascontiguousarray(x[b]), "v": vrow}
        if has_bias:
            m["bias"] = np.ascontiguousarray(bias.reshape(1, S), dtype=np.float32)
        in_maps.append(m)

    res = run_bass_kernel_spmd(
        nc, in_maps, core_ids=list(range(N_CORES)), trace=trace
    )
    out = np.stack([r["out"] for r in res.results], axis=0)
    return out, res


def kernel(x, weight, bias, decay_value):
    out, _ = _run(x, weight, bias, decay_value)
    return out


# revision 27
# speedup vs baseline: 1.1298x; 1.1298x over previous
"""Trainium2 Bass kernel for nn_ColRepeatCausalLinear.

Math: reference computes out = x @ W + bias with
    W[s, t] = v[t] * d^(t-s)  for t >= s, else 0,   d = clip(decay_value, 0.9, 1)
which factorizes as a decayed prefix scan along S:
    y[b, e, t] = d * y[b, e, t-1] + x[b, e, t]
    out[b, e, t] = v[t] * y[b, e, t] + bias[t]
i.e. O(B*E*S) work instead of the O(B*E*S^2) dense matmul.

Mapping: data-parallel over B across 8 NeuronCores (x[b] per core, params
replicated). Per core the kernel sits on the DMA wall: 8 MiB in + 8 MiB
out against a measured ~428 GB/s aggregate DMA fabric (16 engines), so
wall clock ~= (first-byte latency ~8.6us, fixed NEFF preamble + queue
arm) + 16.4 MiB / 428 GB/s + ~2.5us counted epilogue. Structure to stay
at the wall (all measured on HW):
  - x moves in eight [128, S] tiles; each is ONE fully DRAM-contiguous
    1 MiB HWDGE group of exactly 128 x 8 KiB descriptors. Only
    exact-128-descriptor groups spread evenly over all 16 DMA engines
    (64-desc groups ran at half rate; 113/15-desc splits collapsed onto
    one engine, 11x slower).
  - load groups alternate between the two HWDGE queues (SP/Act) and
    each queue gets 4 load + 4 store groups: a queue only sustains ~4
    outstanding trigger groups (a 5th dma_start stalls the issuing
    engine), and alternating makes scan i gate on queue (i%2)'s ~4.7
    us/group pace, so the scan chain advances every ~2.35 us.
  - all load triggers enqueue before any store trigger on both rings
    (an event-gated store descriptor never head-of-line blocks a load);
    each tile's store rides the queue opposite its load, keeping both
    rings supplied through the load->store transition.
  - v is host-cast to bf16 and broadcast across partitions with a K=1
    ones-matmul into PSUM (bf16 moving data runs the PE at 1 cyc/row vs
    4 for fp32, so vb is ready ~2 us after v lands, off the scan gate);
  - the scan+scale runs per e-row tile on the Vector engine via a fused
    custom DVE op (cumsum * v in one pass, ~1 cyc/elem, 2.29us/tile).
Known residual: one DMA engine (ring 15 / engine 79) intermittently
runs ~60% speed; round-robin descriptor assignment has no work
stealing, so such runs straggle ~7-9us draining its share at the end.
No descriptor layout avoids it (see above); this is the bimodality
between ~52.5us and ~60us runs.

Hardcoded problem shapes: x (8, 1024, 2048) f32, weight (1, 2048),
bias (2048,), decay_value (1,).
"""

import numpy as np

import concourse.bacc as bacc
import concourse.mybir as mybir
from concourse.tile import TileContext
from concourse.bass_utils import run_bass_kernel_spmd

B, E, S = 8, 1024, 2048
P = 128
N_CORES = 8
F32 = mybir.dt.float32
BF16 = mybir.dt.bfloat16

_cache = {}

# Fused custom DVE op: out[p,k] = (sum_{j<=k} x[p,j]) * v[p,k] — the whole
# d=1 kernel body in ONE Vector-engine instruction (the stock path needs a
# 2-cyc/elem TensorTensorScan plus a 1-cyc/elem tensor_mul). Registered at
# runtime into dve_ops.OPS; sha self-pinned since this op isn't in-tree.
_FUSED_OP = None
try:
    from concourse import dve_ops as _dops
    from concourse.dve_spec import AluOp as _AluOp, Spec as _Spec
    from concourse.dve_spec import Src0 as _Src0, Src1 as _Src1, scan as _scan
    from concourse.dve_spec import lower as _lower
    from concourse.dve_uop import DveOpSpec as _DveOpSpec

    _FUSED_NAME = "CUMSUM_VSCALE_ANT"
    if _FUSED_NAME in _dops._SUB_OPCODE_FOR_NAME:
        _FUSED_OP = next(o for o in _dops.OPS if o.name == _FUSED_NAME)
    else:
        _fspec = _Spec(body=_scan(_AluOp.ADD, _Src0) * _Src1)
        _row = _dops._CUSTOM_DVE_ROW_BASE + len(_dops.OPS)
        assert _row < 0x20
        _dops._SUB_OPCODE_FOR_NAME[_FUSED_NAME] = _row
        _sha = {}
        for _ver in ("v3", "v4"):
            try:
                _sha[_ver] = _DveOpSpec(
                    name=_FUSED_NAME,
                    opcode=_row,
                    uops=_lower(_fspec, ver=_ver),
                    rd1_en=_dops.has_src1(_fspec),
                ).sha(_ver)
            except Exception:
                pass
        _FUSED_OP = _dops.DveOp(_FUSED_NAME, _fspec, subdim=False, uops_sha=_sha)
        _dops.OPS.append(_FUSED_OP)
        _dops.CUSTOM_DVE_SPECS[_FUSED_NAME] = _fspec
except Exception:
    _FUSED_OP = None

R = 1  # e-rows per partition per tile
BANK = 512  # fp32 elems per PSUM bank


def _build(d: float, has_bias: bool):
    nc = bacc.Bacc(
        "TRN2",
        target_bir_lowering=False,
        debug=False,
        enable_asserts=False,
    )
    x = nc.dram_tensor("x", [E, S], F32, kind="ExternalInput").ap()
    v_dram = nc.dram_tensor("v", [1, S], BF16, kind="ExternalInput").ap()
    bias_dram = None
    if has_bias:
        bias_dram = nc.dram_tensor("bias", [1, S], F32, kind="ExternalInput").ap()
    out = nc.dram_tensor("out", [E, S], F32, kind="ExternalOutput").ap()

    n_tiles = E // (P * R)
    rows = P * R

    with TileContext(nc) as tc:
        with (
            tc.tile_pool(name="const", bufs=1) as cpool,
            tc.tile_pool(name="xs", bufs=n_tiles) as xpool,
            tc.tile_pool(name="ys", bufs=2) as ypool,
            tc.tile_pool(name="os", bufs=n_tiles) as opool,
            tc.tile_pool(name="ps", bufs=1, space="PSUM") as ppool,
        ):
            # v (4 KiB bf16) rides the gpsimd SWDGE queue: slow to start
            # (~13 us arrival) but OFF both HWDGE rings — a leading v
            # group on a bulk ring time-shifts that whole ring by the
            # ~2 us group-handoff latency and made it finish last. With
            # the bf16 PE broadcast (~1.5 us), vb is still ready ~15 us,
            # ahead of every scan/store deadline.
            vrow = cpool.tile([1, S], BF16)
            nc.gpsimd.dma_start(out=vrow[:], in_=v_dram)
            if has_bias:
                brow = cpool.tile([1, S], F32)
                nc.gpsimd.dma_start(out=brow[:], in_=bias_dram)

            # x loads: one fully DRAM-contiguous 1 MiB group (128 x 8 KiB
            # descriptors — only exact-128 groups spread evenly over all 16
            # DMA engines) per [128, S] tile, alternating queues so scan i
            # gates on queue (i%2)'s ~4.7 us/group pace — interleaved, the
            # chain advances every ~2.35 us. 4 load + 4 store groups per
            # queue matches the ~4-outstanding-group HWDGE trigger limit.
            xts = []
            for i in range(n_tiles):
                xt = xpool.tile([P, R * S], F32)
                src = x[i * rows : (i + 1) * rows, :].rearrange(
                    "(p b) s -> p (b s)", b=R
                )
                (nc.scalar if i % 2 == 0 else nc.sync).dma_start(out=xt[:], in_=src)
                xts.append(xt)

            # Broadcast v across partitions with a K=1 matmul against a
            # ones row (out[p, t] = v[t]); bf16 moving data streams the PE
            # at 1 cyc/row. The Vector engine reads vb from PSUM.
            ones = cpool.tile([1, P], BF16)
            nc.vector.memset(ones[:], 1.0)
            vb = ppool.tile([P, S], F32)
            for n in range(S // BANK):
                nc.tensor.matmul(
                    vb[:, n * BANK : (n + 1) * BANK],
                    ones[:],
                    vrow[:, n * BANK : (n + 1) * BANK],
                    start=True,
                    stop=True,
                )
            if has_bias:
                onesf = cpool.tile([1, P], F32)
                nc.vector.memset(onesf[:], 1.0)
                bb = ppool.tile([P, S], F32)
                for n in range(S // BANK):
                    nc.tensor.matmul(
                        bb[:, n * BANK : (n + 1) * BANK],
                        onesf[:],
                        brow[:, n * BANK : (n + 1) * BANK],
                        start=True,
                        stop=True,
                    )
            if not (d == 1.0 and _FUSED_OP is not None):
                dtile = cpool.tile([P, 1], F32)
                nc.gpsimd.memset(dtile[:], d)

            for i in range(n_tiles):
                xt = xts[i]
                ot = opool.tile([P, R * S], F32)
                dst = out[i * rows : (i + 1) * rows, :].rearrange(
                    "(p b) s -> p (b s)", b=R
                )
                for c in range(R):
                    cs = slice(c * S, (c + 1) * S)
                    xc = xt[:, cs]
                    oc = ot[:, cs]
                    if d == 1.0 and _FUSED_OP is not None:
                        nc.vector._custom_dve(_FUSED_OP, out=oc, in0=xc, in1=vb[:])
                    else:
                        yt = ypool.tile([P, S], F32)
                        nc.vector.tensor_tensor_scan(
                            yt[:], dtile[:].broadcast_to([P, S]), xc,
                            0.0, mybir.AluOpType.mult, mybir.AluOpType.add,
                        )
                        nc.vector.tensor_mul(oc, yt[:], vb[:])
                    if has_bias:
                        nc.vector.tensor_add(oc, oc, bb[:])
                # Store the tile as one 1 MiB group on the queue
                # opposite its load.
                (nc.sync if i % 2 == 0 else nc.scalar).dma_start(
                    out=dst, in_=ot[:]
                )
    nc.compile()
    return nc


def _run(x, weight, bias, decay_value, trace=False):
    x = np.asarray(x, dtype=np.float32)
    weight = np.asarray(weight, dtype=np.float32)
    bias = np.asarray(bias, dtype=np.float32)
    decay_value = np.asarray(decay_value)
    assert x.shape == (B, E, S), x.shape

    # DECAY_CONSTANT = 1.0 in the reference; exponent is (t - s) / 1.0.
    d = float(np.clip(np.float64(decay_value.reshape(-1)[0]), 0.9, 1.0))
    has_bias = bool(np.any(bias))

    key = (d, has_bias)
    if key not in _cache:
        _cache[key] = _build(d, has_bias)
    nc = _cache[key]

    import ml_dtypes

    vrow = np.ascontiguousarray(
        weight.reshape(1, S).astype(ml_dtypes.bfloat16)
    )
    in_maps = []
    for b in range(N_CORES):
        m = {"x": np.ascontiguousarray(x[b]), "v": vrow}
        if has_bias:
            m["bias"] = np.ascontiguousarray(bias.reshape(1, S), dtype=np.float32)
        in_maps.append(m)

    res = run_bass_kernel_spmd(
        nc, in_maps, core_ids=list(range(N_CORES)), trace=trace
    )
    out = np.stack([r["out"] for r in res.results], axis=0)
    return out, res


def kernel(x, weight, bias, decay_value):
    out, _ = _run(x, weight, bias, decay_value)
    return out


# revision 28
# speedup vs baseline: 1.1370x; 1.0063x over previous
"""Trainium2 Bass kernel for nn_ColRepeatCausalLinear.

Math: reference computes out = x @ W + bias with
    W[s, t] = v[t] * d^(t-s)  for t >= s, else 0,   d = clip(decay_value, 0.9, 1)
which factorizes as a decayed prefix scan along S:
    y[b, e, t] = d * y[b, e, t-1] + x[b, e, t]
    out[b, e, t] = v[t] * y[b, e, t] + bias[t]
i.e. O(B*E*S) work instead of the O(B*E*S^2) dense matmul.

Mapping: data-parallel over B across 8 NeuronCores (x[b] per core, params
replicated). Per core the kernel sits on the DMA wall: 8 MiB in + 8 MiB
out against a measured ~428 GB/s aggregate DMA fabric (16 engines).

Measured exec-window anatomy (gauge first_useful->last_useful, good
mode ~52us): the window OPENS at the framework's const-AP memsets
(~5.9us absolute; the 0-5.9us runtime preamble never counts), then
~2.3us of trigger-write + HWDGE queue-arm to first byte, ~41us of DMA
(39.2us at the 428 GB/s cap + ~0.6 ramp + ~1.0 scheduler-hoist/event-
check stall + ~0.5 shallow-ring tail), and a ~8.4us 8-core SPMD exit
handshake (~0.9us/core semaphore propagation) that is invariant to
kernel structure (verified across configs with 2x different DMA-group
counts). Rejected with on-HW measurements: SWDGE store routing (2x),
16 KiB-descriptor full-tile stores, load/store transition desync (5/3
split), 0.5 MiB store quartering, single-queue layouts.
Structure to stay at the wall (all measured on HW):
  - x moves in eight [128, S] tiles; each is ONE fully DRAM-contiguous
    1 MiB HWDGE group of exactly 128 x 8 KiB descriptors. Only
    exact-128-descriptor groups spread evenly over all 16 DMA engines
    (64-desc groups ran at half rate; 113/15-desc splits collapsed onto
    one engine, 11x slower).
  - load groups alternate between the two HWDGE queues (SP/Act) and
    each queue gets 4 load + 4 store groups: a queue only sustains ~4
    outstanding trigger groups (a 5th dma_start stalls the issuing
    engine), and alternating makes scan i gate on queue (i%2)'s ~4.7
    us/group pace, so the scan chain advances every ~2.35 us.
  - all load triggers enqueue before any store trigger on both rings
    (an event-gated store descriptor never head-of-line blocks a load);
    each tile's store rides the queue opposite its load, keeping both
    rings supplied through the load->store transition.
  - v is host-cast to bf16 and broadcast across partitions with a K=1
    ones-matmul into PSUM (bf16 moving data runs the PE at 1 cyc/row vs
    4 for fp32, so vb is ready ~2 us after v lands, off the scan gate);
  - the scan+scale runs per e-row tile on the Vector engine via a fused
    custom DVE op (cumsum * v in one pass, ~1 cyc/elem, 2.29us/tile).
Known residual: one DMA engine (ring 15 / engine 79) intermittently
runs ~60% speed; round-robin descriptor assignment has no work
stealing, so such runs straggle ~7-9us draining its share at the end.
No descriptor layout avoids it (see above); this is the bimodality
between ~52.5us and ~60us runs.

Hardcoded problem shapes: x (8, 1024, 2048) f32, weight (1, 2048),
bias (2048,), decay_value (1,).
"""

import numpy as np

import concourse.bacc as bacc
import concourse.mybir as mybir
from concourse.tile import TileContext
from concourse.bass_utils import run_bass_kernel_spmd

B, E, S = 8, 1024, 2048
P = 128
N_CORES = 8
F32 = mybir.dt.float32
BF16 = mybir.dt.bfloat16

_cache = {}

# Fused custom DVE op: out[p,k] = (sum_{j<=k} x[p,j]) * v[p,k] — the whole
# d=1 kernel body in ONE Vector-engine instruction (the stock path needs a
# 2-cyc/elem TensorTensorScan plus a 1-cyc/elem tensor_mul). Registered at
# runtime into dve_ops.OPS; sha self-pinned since this op isn't in-tree.
_FUSED_OP = None
try:
    from concourse import dve_ops as _dops
    from concourse.dve_spec import AluOp as _AluOp, Spec as _Spec
    from concourse.dve_spec import Src0 as _Src0, Src1 as _Src1, scan as _scan
    from concourse.dve_spec import lower as _lower
    from concourse.dve_uop import DveOpSpec as _DveOpSpec

    _FUSED_NAME = "CUMSUM_VSCALE_ANT"
    if _FUSED_NAME in _dops._SUB_OPCODE_FOR_NAME:
        _FUSED_OP = next(o for o in _dops.OPS if o.name == _FUSED_NAME)
    else:
        _fspec = _Spec(body=_scan(_AluOp.ADD, _Src0) * _Src1)
        _row = _dops._CUSTOM_DVE_ROW_BASE + len(_dops.OPS)
        assert _row < 0x20
        _dops._SUB_OPCODE_FOR_NAME[_FUSED_NAME] = _row
        _sha = {}
        for _ver in ("v3", "v4"):
            try:
                _sha[_ver] = _DveOpSpec(
                    name=_FUSED_NAME,
                    opcode=_row,
                    uops=_lower(_fspec, ver=_ver),
                    rd1_en=_dops.has_src1(_fspec),
                ).sha(_ver)
            except Exception:
                pass
        _FUSED_OP = _dops.DveOp(_FUSED_NAME, _fspec, subdim=False, uops_sha=_sha)
        _dops.OPS.append(_FUSED_OP)
        _dops.CUSTOM_DVE_SPECS[_FUSED_NAME] = _fspec
except Exception:
    _FUSED_OP = None

R = 1  # e-rows per partition per tile
BANK = 512  # fp32 elems per PSUM bank


def _build(d: float, has_bias: bool):
    nc = bacc.Bacc(
        "TRN2",
        target_bir_lowering=False,
        debug=False,
        enable_asserts=False,
    )
    x = nc.dram_tensor("x", [E, S], F32, kind="ExternalInput").ap()
    v_dram = nc.dram_tensor("v", [1, S], BF16, kind="ExternalInput").ap()
    bias_dram = None
    if has_bias:
        bias_dram = nc.dram_tensor("bias", [1, S], F32, kind="ExternalInput").ap()
    out = nc.dram_tensor("out", [E, S], F32, kind="ExternalOutput").ap()

    n_tiles = E // (P * R)
    rows = P * R

    with TileContext(nc) as tc:
        with (
            tc.tile_pool(name="const", bufs=1) as cpool,
            tc.tile_pool(name="xs", bufs=n_tiles) as xpool,
            tc.tile_pool(name="ys", bufs=2) as ypool,
            tc.tile_pool(name="os", bufs=n_tiles) as opool,
            tc.tile_pool(name="ps", bufs=1, space="PSUM") as ppool,
        ):
            # v (4 KiB bf16) rides the gpsimd SWDGE queue: slow to start
            # (~13 us arrival) but OFF both HWDGE rings — a leading v
            # group on a bulk ring time-shifts that whole ring by the
            # ~2 us group-handoff latency and made it finish last. With
            # the bf16 PE broadcast (~1.5 us), vb is still ready ~15 us,
            # ahead of every scan/store deadline.
            vrow = cpool.tile([1, S], BF16)
            nc.gpsimd.dma_start(out=vrow[:], in_=v_dram)
            if has_bias:
                brow = cpool.tile([1, S], F32)
                nc.gpsimd.dma_start(out=brow[:], in_=bias_dram)

            # x loads: one fully DRAM-contiguous 1 MiB group (128 x 8 KiB
            # descriptors — only exact-128 groups spread evenly over all 16
            # DMA engines) per [128, S] tile, alternating queues so scan i
            # gates on queue (i%2)'s ~4.7 us/group pace — interleaved, the
            # chain advances every ~2.35 us. 4 load + 4 store groups per
            # queue matches the ~4-outstanding-group HWDGE trigger limit.
            xts = []
            for i in range(n_tiles):
                xt = xpool.tile([P, R * S], F32)
                src = x[i * rows : (i + 1) * rows, :].rearrange(
                    "(p b) s -> p (b s)", b=R
                )
                (nc.scalar if i % 2 == 0 else nc.sync).dma_start(out=xt[:], in_=src)
                xts.append(xt)

            # Broadcast v across partitions with a K=1 matmul against a
            # ones row (out[p, t] = v[t]); bf16 moving data streams the PE
            # at 1 cyc/row. The Vector engine reads vb from PSUM.
            ones = cpool.tile([1, P], BF16)
            nc.vector.memset(ones[:], 1.0)
            vb = ppool.tile([P, S], F32)
            for n in range(S // BANK):
                nc.tensor.matmul(
                    vb[:, n * BANK : (n + 1) * BANK],
                    ones[:],
                    vrow[:, n * BANK : (n + 1) * BANK],
                    start=True,
                    stop=True,
                )
            if has_bias:
                onesf = cpool.tile([1, P], F32)
                nc.vector.memset(onesf[:], 1.0)
                bb = ppool.tile([P, S], F32)
                for n in range(S // BANK):
                    nc.tensor.matmul(
                        bb[:, n * BANK : (n + 1) * BANK],
                        onesf[:],
                        brow[:, n * BANK : (n + 1) * BANK],
                        start=True,
                        stop=True,
                    )
            if not (d == 1.0 and _FUSED_OP is not None):
                dtile = cpool.tile([P, 1], F32)
                nc.gpsimd.memset(dtile[:], d)

            for i in range(n_tiles):
                xt = xts[i]
                ot = opool.tile([P, R * S], F32)
                dst = out[i * rows : (i + 1) * rows, :].rearrange(
                    "(p b) s -> p (b s)", b=R
                )
                for c in range(R):
                    cs = slice(c * S, (c + 1) * S)
                    xc = xt[:, cs]
                    oc = ot[:, cs]
                    if d == 1.0 and _FUSED_OP is not None:
                        nc.vector._custom_dve(_FUSED_OP, out=oc, in0=xc, in1=vb[:])
                    else:
                        yt = ypool.tile([P, S], F32)
                        nc.vector.tensor_tensor_scan(
                            yt[:], dtile[:].broadcast_to([P, S]), xc,
                            0.0, mybir.AluOpType.mult, mybir.AluOpType.add,
                        )
                        nc.vector.tensor_mul(oc, yt[:], vb[:])
                    if has_bias:
                        nc.vector.tensor_add(oc, oc, bb[:])
                # Store the tile as one 1 MiB group on the queue
                # opposite its load.
                (nc.sync if i % 2 == 0 else nc.scalar).dma_start(
                    out=dst, in_=ot[:]
                )
    nc.compile()
    return nc


def _run(x, weight, bias, decay_value, trace=False):
    x = np.asarray(x, dtype=np.float32)
    weight = np.asarray(weight, dtype=np.float32)
    bias = np.asarray(bias, dtype=np.float32)
    decay_value = np.asarray(decay_value)
    assert x.shape == (B, E, S), x.shape

    # DECAY_CONSTANT = 1.0 in the reference; exponent is (t - s) / 1.0.
    d = float(np.clip(np.float64(decay_value.reshape(-1)[0]), 0.9, 1.0))
    has_bias = bool(np.any(bias))

    key = (d, has_bias)
    if key not in _cache:
        _cache[key] = _build(d, has_bias)
    nc = _cache[key]

    import ml_dtypes

    vrow = np.ascontiguousarray(
        weight.reshape(1, S).astype(ml_dtypes.bfloat16)
    )
    in_maps = []
    for b in range(N_CORES):
        m = {"x": np.ascontiguousarray(x[b]), "v": vrow}
        if has_bias:
            m["bias"] = np.ascontiguousarray(bias.reshape(1, S), dtype=np.float32)
        in_maps.append(m)

    res = run_bass_kernel_spmd(
        nc, in_maps, core_ids=list(range(N_CORES)), trace=trace
    )
    out = np.stack([r["out"] for r in res.results], axis=0)
    return out, res


def kernel(x, weight, bias, decay_value):
    out, _ = _run(x, weight, bias, decay_value)
    return out


# revision 30
# speedup vs baseline: 1.1409x; 1.0034x over previous
"""Trainium2 Bass kernel for nn_ColRepeatCausalLinear.

Math: reference computes out = x @ W + bias with
    W[s, t] = v[t] * d^(t-s)  for t >= s, else 0,   d = clip(decay_value, 0.9, 1)
which factorizes as a decayed prefix scan along S:
    y[b, e, t] = d * y[b, e, t-1] + x[b, e, t]
    out[b, e, t] = v[t] * y[b, e, t] + bias[t]
i.e. O(B*E*S) work instead of the O(B*E*S^2) dense matmul.

Mapping: data-parallel over B across 8 NeuronCores (x[b] per core, params
replicated). Per core the kernel sits on the DMA wall: 8 MiB in + 8 MiB
out against a measured ~428 GB/s aggregate DMA fabric (16 engines).

Measured exec-window anatomy (gauge first_useful->last_useful, good
mode ~52us): the window OPENS at the framework's const-AP memsets
(~5.9us absolute; the 0-5.9us runtime preamble never counts), then
~2.3us of trigger-write + HWDGE queue-arm to first byte, ~41us of DMA
(39.2us at the 428 GB/s cap + ~0.6 ramp + ~1.0 scheduler-hoist/event-
check stall + ~0.5 shallow-ring tail), and a ~8.4us 8-core SPMD exit
handshake (~0.9us/core semaphore propagation) that is invariant to
kernel structure (verified across configs with 2x different DMA-group
counts). Rejected with on-HW measurements: SWDGE store routing (2x),
16 KiB-descriptor full-tile stores, load/store transition desync (5/3
split), 0.5 MiB store quartering, single-queue layouts.
Structure to stay at the wall (all measured on HW):
  - x moves in eight [128, S] tiles; each is ONE fully DRAM-contiguous
    1 MiB HWDGE group of exactly 128 x 8 KiB descriptors. Only
    exact-128-descriptor groups spread evenly over all 16 DMA engines
    (64-desc groups ran at half rate; 113/15-desc splits collapsed onto
    one engine, 11x slower).
  - load groups alternate between the two HWDGE queues (SP/Act) and
    each queue gets 4 load + 4 store groups: a queue only sustains ~4
    outstanding trigger groups (a 5th dma_start stalls the issuing
    engine), and alternating makes scan i gate on queue (i%2)'s ~4.7
    us/group pace, so the scan chain advances every ~2.35 us.
  - all load triggers enqueue before any store trigger on both rings
    (an event-gated store descriptor never head-of-line blocks a load);
    each tile's store rides the queue opposite its load, keeping both
    rings supplied through the load->store transition.
  - v is host-cast to bf16 and broadcast across partitions with a K=1
    ones-matmul into PSUM (bf16 moving data runs the PE at 1 cyc/row vs
    4 for fp32, so vb is ready ~2 us after v lands, off the scan gate);
  - the scan+scale runs per e-row tile on the Vector engine via a fused
    custom DVE op (cumsum * v in one pass, ~1 cyc/elem, 2.29us/tile).
Known residual: one DMA engine (ring 15 / engine 79) intermittently
runs ~60% speed; round-robin descriptor assignment has no work
stealing, so such runs straggle ~7-9us draining its share at the end.
No descriptor layout avoids it (see above); this is the bimodality
between ~52.5us and ~60us runs.

Hardcoded problem shapes: x (8, 1024, 2048) f32, weight (1, 2048),
bias (2048,), decay_value (1,).
"""

import numpy as np

import concourse.bacc as bacc
import concourse.mybir as mybir
from concourse.tile import TileContext
from concourse.bass_utils import run_bass_kernel_spmd

B, E, S = 8, 1024, 2048
P = 128
N_CORES = 8
F32 = mybir.dt.float32
BF16 = mybir.dt.bfloat16

_cache = {}

# Fused custom DVE op: out[p,k] = (sum_{j<=k} x[p,j]) * v[p,k] — the whole
# d=1 kernel body in ONE Vector-engine instruction (the stock path needs a
# 2-cyc/elem TensorTensorScan plus a 1-cyc/elem tensor_mul). Registered at
# runtime into dve_ops.OPS; sha self-pinned since this op isn't in-tree.
_FUSED_OP = None
try:
    from concourse import dve_ops as _dops
    from concourse.dve_spec import AluOp as _AluOp, Spec as _Spec
    from concourse.dve_spec import Src0 as _Src0, Src1 as _Src1, scan as _scan
    from concourse.dve_spec import lower as _lower
    from concourse.dve_uop import DveOpSpec as _DveOpSpec

    _FUSED_NAME = "CUMSUM_VSCALE_ANT"
    if _FUSED_NAME in _dops._SUB_OPCODE_FOR_NAME:
        _FUSED_OP = next(o for o in _dops.OPS if o.name == _FUSED_NAME)
    else:
        _fspec = _Spec(body=_scan(_AluOp.ADD, _Src0) * _Src1)
        _row = _dops._CUSTOM_DVE_ROW_BASE + len(_dops.OPS)
        assert _row < 0x20
        _dops._SUB_OPCODE_FOR_NAME[_FUSED_NAME] = _row
        _sha = {}
        for _ver in ("v3", "v4"):
            try:
                _sha[_ver] = _DveOpSpec(
                    name=_FUSED_NAME,
                    opcode=_row,
                    uops=_lower(_fspec, ver=_ver),
                    rd1_en=_dops.has_src1(_fspec),
                ).sha(_ver)
            except Exception:
                pass
        _FUSED_OP = _dops.DveOp(_FUSED_NAME, _fspec, subdim=False, uops_sha=_sha)
        _dops.OPS.append(_FUSED_OP)
        _dops.CUSTOM_DVE_SPECS[_FUSED_NAME] = _fspec
except Exception:
    _FUSED_OP = None

R = 1  # e-rows per partition per tile
BANK = 512  # fp32 elems per PSUM bank


def _build(d: float, has_bias: bool):
    nc = bacc.Bacc(
        "TRN2",
        target_bir_lowering=False,
        debug=False,
        enable_asserts=False,
    )
    x = nc.dram_tensor("x", [E, S], F32, kind="ExternalInput").ap()
    v_dram = nc.dram_tensor("v", [1, S], BF16, kind="ExternalInput").ap()
    bias_dram = None
    if has_bias:
        bias_dram = nc.dram_tensor("bias", [1, S], F32, kind="ExternalInput").ap()
    out = nc.dram_tensor("out", [E, S], F32, kind="ExternalOutput").ap()

    n_tiles = E // (P * R)
    rows = P * R

    with TileContext(nc) as tc:
        with (
            tc.tile_pool(name="const", bufs=1) as cpool,
            tc.tile_pool(name="xs", bufs=n_tiles) as xpool,
            tc.tile_pool(name="ys", bufs=2) as ypool,
            tc.tile_pool(name="os", bufs=n_tiles) as opool,
            tc.tile_pool(name="ps", bufs=1, space="PSUM") as ppool,
        ):
            # v (4 KiB bf16) rides the gpsimd SWDGE queue: slow to start
            # (~13 us arrival) but OFF both HWDGE rings — a leading v
            # group on a bulk ring time-shifts that whole ring by the
            # ~2 us group-handoff latency and made it finish last. With
            # the bf16 PE broadcast (~1.5 us), vb is still ready ~15 us,
            # ahead of every scan/store deadline.
            vrow = cpool.tile([1, S], BF16)
            nc.gpsimd.dma_start(out=vrow[:], in_=v_dram)
            if has_bias:
                brow = cpool.tile([1, S], F32)
                nc.gpsimd.dma_start(out=brow[:], in_=bias_dram)

            # x loads: one fully DRAM-contiguous 1 MiB group (128 x 8 KiB
            # descriptors — only exact-128 groups spread evenly over all 16
            # DMA engines) per [128, S] tile, alternating queues so scan i
            # gates on queue (i%2)'s ~4.7 us/group pace — interleaved, the
            # chain advances every ~2.35 us. 4 load + 4 store groups per
            # queue matches the ~4-outstanding-group HWDGE trigger limit.
            xts = []
            for i in range(n_tiles):
                xt = xpool.tile([P, R * S], F32)
                src = x[i * rows : (i + 1) * rows, :].rearrange(
                    "(p b) s -> p (b s)", b=R
                )
                (nc.scalar if i % 2 == 0 else nc.sync).dma_start(out=xt[:], in_=src)
                xts.append(xt)

            # Broadcast v across partitions with a K=1 matmul against a
            # ones row (out[p, t] = v[t]); bf16 moving data streams the PE
            # at 1 cyc/row. The Vector engine reads vb from PSUM.
            ones = cpool.tile([1, P], BF16)
            nc.vector.memset(ones[:], 1.0)
            vb = ppool.tile([P, S], F32)
            for n in range(S // BANK):
                nc.tensor.matmul(
                    vb[:, n * BANK : (n + 1) * BANK],
                    ones[:],
                    vrow[:, n * BANK : (n + 1) * BANK],
                    start=True,
                    stop=True,
                )
            if has_bias:
                onesf = cpool.tile([1, P], F32)
                nc.vector.memset(onesf[:], 1.0)
                bb = ppool.tile([P, S], F32)
                for n in range(S // BANK):
                    nc.tensor.matmul(
                        bb[:, n * BANK : (n + 1) * BANK],
                        onesf[:],
                        brow[:, n * BANK : (n + 1) * BANK],
                        start=True,
                        stop=True,
                    )
            if not (d == 1.0 and _FUSED_OP is not None):
                dtile = cpool.tile([P, 1], F32)
                nc.gpsimd.memset(dtile[:], d)

            for i in range(n_tiles):
                xt = xts[i]
                ot = opool.tile([P, R * S], F32)
                dst = out[i * rows : (i + 1) * rows, :].rearrange(
                    "(p b) s -> p (b s)", b=R
                )
                for c in range(R):
                    cs = slice(c * S, (c + 1) * S)
                    xc = xt[:, cs]
                    oc = ot[:, cs]
                    if d == 1.0 and _FUSED_OP is not None:
                        nc.vector._custom_dve(_FUSED_OP, out=oc, in0=xc, in1=vb[:])
                    else:
                        yt = ypool.tile([P, S], F32)
                        nc.vector.tensor_tensor_scan(
                            yt[:], dtile[:].broadcast_to([P, S]), xc,
                            0.0, mybir.AluOpType.mult, mybir.AluOpType.add,
                        )
                        nc.vector.tensor_mul(oc, yt[:], vb[:])
                    if has_bias:
                        nc.vector.tensor_add(oc, oc, bb[:])
                # Store the tile as one 1 MiB group on the queue
                # opposite its load.
                (nc.sync if i % 2 == 0 else nc.scalar).dma_start(
                    out=dst, in_=ot[:]
                )
    nc.compile()
    return nc


def _run(x, weight, bias, decay_value, trace=False):
    x = np.asarray(x, dtype=np.float32)
    weight = np.asarray(weight, dtype=np.float32)
    bias = np.asarray(bias, dtype=np.float32)
    decay_value = np.asarray(decay_value)
    assert x.shape == (B, E, S), x.shape

    # DECAY_CONSTANT = 1.0 in the reference; exponent is (t - s) / 1.0.
    d = float(np.clip(np.float64(decay_value.reshape(-1)[0]), 0.9, 1.0))
    has_bias = bool(np.any(bias))

    key = (d, has_bias)
    if key not in _cache:
        _cache[key] = _build(d, has_bias)
    nc = _cache[key]

    import ml_dtypes

    vrow = np.ascontiguousarray(
        weight.reshape(1, S).astype(ml_dtypes.bfloat16)
    )
    in_maps = []
    for b in range(N_CORES):
        m = {"x": np.ascontiguousarray(x[b]), "v": vrow}
        if has_bias:
            m["bias"] = np.ascontiguousarray(bias.reshape(1, S), dtype=np.float32)
        in_maps.append(m)

    res = run_bass_kernel_spmd(
        nc, in_maps, core_ids=list(range(N_CORES)), trace=trace
    )
    out = np.stack([r["out"] for r in res.results], axis=0)
    return out, res


def kernel(x, weight, bias, decay_value):
    out, _ = _run(x, weight, bias, decay_value)
    return out
